# revision 27
# baseline (speedup 1.0000x reference)
"""Continuous Wavelet Transform (4-scale Morlet, 129-tap) on 8 TRN2 NeuronCores.

The reference pads H and W by 3 and crops back after a conv along W — the
pad/crop cancels exactly, so the whole module reduces to a SAME 129-tap
correlation of each of the B*C*H rows with 4 wavelet kernels.

Strategy (data-parallel over B, one batch element per core):
  out[w] = sum_k ker[k] * x[w + k - 64]
With x zero-padded by 64 on each side (X, length 1152) and tiled in 128-wide
tiles XT_m, each 128-wide output tile j is exactly two matmuls:
  out_j[q] = sum_p XT_j[p] * P[p,q] + sum_p XT_{j+1}[p] * Q[p,q]
  P[p,q] = ker[p-q]     (p >= q, lower-triangular Toeplitz)
  Q[p,q] = ker[128+p-q] (p <= q, upper-triangular Toeplitz)
The 4 scales are concatenated along the moving free dim (4*128 = 512 cols =
one PSUM bank). x is transposed/padded/bf16-cast on the host so the device
sees [position, row] layout directly (TensorE contracts over partitions).
"""
import numpy as np
import ml_dtypes

import concourse.bacc as bacc
import concourse.mybir as mybir
import concourse.tile as tile
from concourse.bass_utils import run_bass_kernel_spmd

BF16 = ml_dtypes.bfloat16
N_CORES = 8
B, C, H, W = 8, 16, 128, 1024
S = 4
SCALES = (2.0, 4.0, 8.0, 16.0)
MORLET_W0 = 5.0
ROWS = C * H              # 2048 rows per core
CHUNKS = ROWS // 128      # 16 row-chunks
JT = W // 128             # 8 output W-tiles
MT = JT + 1               # 9 stationary x tiles

COMPUTE_DT = mybir.dt.bfloat16
COMPUTE_NP = BF16

GROUPS = 8                     # row groups per core
GROUP_ROWS = ROWS // GROUPS    # 256 rows per group (2 chunks)
CHUNKS_PER_GROUP = GROUP_ROWS // 128


def _wavelet_bank():
    t = np.arange(-64, 65, dtype=np.float32)  # [129]
    return np.stack([
        np.exp(-0.5 * (t / s) ** 2) * np.cos(MORLET_W0 * t / s) / np.sqrt(s)
        for s in SCALES
    ]).astype(np.float32)  # [S, 129]


def _weights():
    """WP, WQ: [128, S*128] with WP[p, s*128+q] = P_s[p,q], same for Q."""
    bank = _wavelet_bank()
    p, q = np.indices((128, 128))
    WP = np.zeros((128, S * 128), np.float32)
    WQ = np.zeros((128, S * 128), np.float32)
    for s in range(S):
        Ps = np.where(p >= q, bank[s][(p - q) % 129], 0.0)
        Qs = np.where(p <= q, bank[s][(128 + p - q) % 129], 0.0)
        WP[:, s * 128:(s + 1) * 128] = Ps
        WQ[:, s * 128:(s + 1) * 128] = Qs
    return WP.astype(COMPUTE_NP), WQ.astype(COMPUTE_NP)


def _build_nc(reps=1, out_split=True, psum_bufs=6, xpool_bufs=5,
              tail_quarters=2):
    nc = bacc.Bacc("TRN2", target_bir_lowering=False, debug=False,
                   num_devices=N_CORES)
    # xt[g, m, p, c]: row-group, x-tile, position-in-tile, row-in-group
    xt_d = nc.declare_dram_parameter("xt", [GROUPS, MT, 128, GROUP_ROWS],
                                     COMPUTE_DT, isOutput=False)
    # wpq = [WP | WQ] along free dim
    wpq_d = nc.declare_dram_parameter("wpq", [128, 2 * S * 128], COMPUTE_DT,
                                      isOutput=False)
    # out[r, h, j, s*128+q]: chunk-r (=channel), H, W-tile, scale-block
    out_d = nc.declare_dram_parameter("out", [CHUNKS, 128, JT * S * 128],
                                      COMPUTE_DT, isOutput=True)

    f32 = mybir.dt.float32
    with tile.TileContext(nc) as tc:
        with (
            tc.tile_pool(name="consts", bufs=1) as consts,
            tc.tile_pool(name="xpool", bufs=xpool_bufs) as xpool,
            tc.tile_pool(name="opool", bufs=3) as opool,
            tc.tile_pool(name="psum", bufs=psum_bufs, space="PSUM") as psum_pool,
        ):
            def chunk_body(r, lhs_of_m, last_chunk):
                outbuf = opool.tile([128, JT * S * 128], COMPUTE_DT,
                                    name="outbuf", tag="outbuf")
                ps = [None] * JT
                for m in range(MT):
                    lhs = lhs_of_m(m)
                    if m < JT:
                        ps[m] = psum_pool.tile([128, S * 128], f32,
                                               name="ps", tag="ps")
                        nc.tensor.matmul(ps[m][:], lhs, wp[:],
                                         start=True, stop=False)
                    if m >= 1:
                        j = m - 1
                        nc.tensor.matmul(ps[j][:], lhs, wq[:],
                                         start=False, stop=True)
                        dst = outbuf[:, j * 512:(j + 1) * 512]
                        if j % 2 == 0:
                            nc.scalar.copy(dst, ps[j][:])
                        else:
                            nc.vector.tensor_copy(dst, ps[j][:])
                        if last_chunk:
                            # quarter-granularity drain of the final chunk
                            if j in (1, 3, 5):
                                nc.sync.dma_start(
                                    out_d[r, :, (j - 1) * 512:(j + 1) * 512],
                                    outbuf[:, (j - 1) * 512:(j + 1) * 512])
                        elif j == 3 and out_split:
                            nc.sync.dma_start(out_d[r, :, 0:2048],
                                              outbuf[:, 0:2048])
                if last_chunk:
                    nc.sync.dma_start(out_d[r, :, 3 * 1024:4096],
                                      outbuf[:, 3 * 1024:4096])
                elif out_split:
                    nc.sync.dma_start(out_d[r, :, 2048:4096],
                                      outbuf[:, 2048:4096])
                else:
                    nc.sync.dma_start(out_d[r], outbuf[:])

            wp = consts.tile([128, S * 128], COMPUTE_DT)
            wq = consts.tile([128, S * 128], COMPUTE_DT)

            for rep in range(reps):
                for g in range(GROUPS):
                    xt = xpool.tile([128, MT, GROUP_ROWS], COMPUTE_DT,
                                    name="xt", tag="xt")
                    # input prefetch on ACT HWDGE ring, separate from the
                    # output DMAs on the SP ring
                    nc.scalar.dma_start(xt[:],
                                        xt_d[g].rearrange("m p c -> p m c"))
                    if rep == 0 and g == 0:
                        # after the first input group so they don't delay it
                        nc.sync.dma_start(wp[:], wpq_d[:, 0:512])
                        nc.sync.dma_start(wq[:], wpq_d[:, 512:1024])
                    for half in range(CHUNKS_PER_GROUP):
                        r = g * CHUNKS_PER_GROUP + half
                        cs = slice(half * 128, (half + 1) * 128)
                        chunk_body(r, lambda m, cs=cs: xt[:, m, cs],
                                   r == CHUNKS - 1)
    nc.compile()
    return nc


_NC_CACHE = {}


def _get_nc(reps=1):
    if reps not in _NC_CACHE:
        _NC_CACHE[reps] = _build_nc(reps)
    return _NC_CACHE[reps]


def _prep_core_input(xb):
    """xb: [C, H, W] float32 -> dict of device input arrays (bf16).

    xt[g, m, p, c] = X[128m+p, 256g+c] where X = x.T zero-padded by 64."""
    rows = np.ascontiguousarray(xb.reshape(ROWS, W))
    X = np.zeros((MT * 128, ROWS), dtype=COMPUTE_NP)
    X[64:64 + W, :] = rows.T.astype(COMPUTE_NP)
    xt = X.reshape(MT, 128, GROUPS, GROUP_ROWS)
    return {"xt": np.ascontiguousarray(xt.transpose(2, 0, 1, 3))}


def _in_maps(x):
    WP, WQ = _weights()
    wpq = np.ascontiguousarray(np.concatenate([WP, WQ], axis=1))
    return [dict(_prep_core_input(x[b]), wpq=wpq) for b in range(N_CORES)]


def _postprocess(out_dev):
    """out_dev: [CHUNKS, 128, JT*S*128] bf16 -> [C, S, H, W] float32."""
    o = np.asarray(out_dev).astype(np.float32)
    o = o.reshape(C, 128, JT, S, 128).transpose(0, 3, 1, 2, 4)
    return o.reshape(C, S, H, W)


def kernel(x):
    x = np.asarray(x, dtype=np.float32)
    assert x.shape == (B, C, H, W)
    in_maps = _in_maps(x)
    nc = _get_nc()
    res = run_bass_kernel_spmd(nc, in_maps, core_ids=list(range(N_CORES)))
    out = np.stack([_postprocess(res.results[b]["out"]) for b in range(N_CORES)])
    return out  # [B, C, S, H, W] float32


# revision 28
# speedup vs baseline: 3.8188x; 3.8188x over previous
"""Continuous Wavelet Transform (4-scale Morlet, 129-tap) on 8 TRN2 NeuronCores.

The reference pads H and W by 3 and crops back after a conv along W — the
pad/crop cancels exactly, so the whole module reduces to a SAME 129-tap
correlation of each of the B*C*H rows with 4 wavelet kernels.

Strategy (data-parallel over B, one batch element per core):
  out[w] = sum_k ker[k] * x[w + k - 64]
With x zero-padded by 64 on each side (X, length 1152) and tiled in 128-wide
tiles XT_m, each 128-wide output tile j is exactly two matmuls:
  out_j[q] = sum_p XT_j[p] * P[p,q] + sum_p XT_{j+1}[p] * Q[p,q]
  P[p,q] = ker[p-q]     (p >= q, lower-triangular Toeplitz)
  Q[p,q] = ker[128+p-q] (p <= q, upper-triangular Toeplitz)
The 4 scales are concatenated along the moving free dim (4*128 = 512 cols =
one PSUM bank). x is transposed/padded/bf16-cast on the host so the device
sees [position, row] layout directly (TensorE contracts over partitions).
"""
import numpy as np
import ml_dtypes

import concourse.bacc as bacc
import concourse.mybir as mybir
import concourse.tile as tile
from concourse.bass_utils import run_bass_kernel_spmd

BF16 = ml_dtypes.bfloat16
N_CORES = 8
B, C, H, W = 8, 16, 128, 1024
S = 4
SCALES = (2.0, 4.0, 8.0, 16.0)
MORLET_W0 = 5.0
ROWS = C * H              # 2048 rows per core
CHUNKS = ROWS // 128      # 16 row-chunks
JT = W // 128             # 8 output W-tiles
MT = JT + 1               # 9 stationary x tiles

COMPUTE_DT = mybir.dt.bfloat16
COMPUTE_NP = BF16

GROUPS = 8                     # row groups per core
GROUP_ROWS = ROWS // GROUPS    # 256 rows per group (2 chunks)
CHUNKS_PER_GROUP = GROUP_ROWS // 128


def _wavelet_bank():
    t = np.arange(-64, 65, dtype=np.float32)  # [129]
    return np.stack([
        np.exp(-0.5 * (t / s) ** 2) * np.cos(MORLET_W0 * t / s) / np.sqrt(s)
        for s in SCALES
    ]).astype(np.float32)  # [S, 129]


def _weights():
    """WP, WQ: [128, S*128] with WP[p, s*128+q] = P_s[p,q], same for Q."""
    bank = _wavelet_bank()
    p, q = np.indices((128, 128))
    WP = np.zeros((128, S * 128), np.float32)
    WQ = np.zeros((128, S * 128), np.float32)
    for s in range(S):
        Ps = np.where(p >= q, bank[s][(p - q) % 129], 0.0)
        Qs = np.where(p <= q, bank[s][(128 + p - q) % 129], 0.0)
        WP[:, s * 128:(s + 1) * 128] = Ps
        WQ[:, s * 128:(s + 1) * 128] = Qs
    return WP.astype(COMPUTE_NP), WQ.astype(COMPUTE_NP)


def _build_nc(reps=1, out_split=True, psum_bufs=6, xpool_bufs=5):
    nc = bacc.Bacc("TRN2", target_bir_lowering=False, debug=False,
                   num_devices=N_CORES)
    # xt[g, m, p, c]: row-group, x-tile, position-in-tile, row-in-group
    xt_d = nc.declare_dram_parameter("xt", [GROUPS, MT, 128, GROUP_ROWS],
                                     COMPUTE_DT, isOutput=False)
    # wpq = [WP | WQ] along free dim
    wpq_d = nc.declare_dram_parameter("wpq", [128, 2 * S * 128], COMPUTE_DT,
                                      isOutput=False)
    # out[r, h, j, s*128+q]: chunk-r (=channel), H, W-tile, scale-block
    out_d = nc.declare_dram_parameter("out", [CHUNKS, 128, JT * S * 128],
                                      COMPUTE_DT, isOutput=True)

    f32 = mybir.dt.float32
    with tile.TileContext(nc) as tc:
        with (
            tc.tile_pool(name="consts", bufs=1) as consts,
            tc.tile_pool(name="xpool", bufs=xpool_bufs) as xpool,
            tc.tile_pool(name="opool", bufs=3) as opool,
            tc.tile_pool(name="psum", bufs=psum_bufs, space="PSUM") as psum_pool,
        ):
            def chunk_body(r, lhs_of_m, last_chunk):
                outbuf = opool.tile([128, JT * S * 128], COMPUTE_DT,
                                    name="outbuf", tag="outbuf")
                ps = [None] * JT
                for m in range(MT):
                    lhs = lhs_of_m(m)
                    if m < JT:
                        ps[m] = psum_pool.tile([128, S * 128], f32,
                                               name="ps", tag="ps")
                        nc.tensor.matmul(ps[m][:], lhs, wp[:],
                                         start=True, stop=False)
                    if m >= 1:
                        j = m - 1
                        nc.tensor.matmul(ps[j][:], lhs, wq[:],
                                         start=False, stop=True)
                        dst = outbuf[:, j * 512:(j + 1) * 512]
                        if j % 2 == 0:
                            nc.scalar.copy(dst, ps[j][:])
                        else:
                            nc.vector.tensor_copy(dst, ps[j][:])
                        if last_chunk:
                            # quarter-granularity drain of the final chunk
                            if j in (1, 3, 5):
                                nc.sync.dma_start(
                                    out_d[r, :, (j - 1) * 512:(j + 1) * 512],
                                    outbuf[:, (j - 1) * 512:(j + 1) * 512])
                        elif j == 3 and out_split:
                            nc.sync.dma_start(out_d[r, :, 0:2048],
                                              outbuf[:, 0:2048])
                if last_chunk:
                    nc.sync.dma_start(out_d[r, :, 3 * 1024:4096],
                                      outbuf[:, 3 * 1024:4096])
                elif out_split:
                    nc.sync.dma_start(out_d[r, :, 2048:4096],
                                      outbuf[:, 2048:4096])
                else:
                    nc.sync.dma_start(out_d[r], outbuf[:])

            wp = consts.tile([128, S * 128], COMPUTE_DT)
            wq = consts.tile([128, S * 128], COMPUTE_DT)

            for rep in range(reps):
                for g in range(GROUPS):
                    xt = xpool.tile([128, MT, GROUP_ROWS], COMPUTE_DT,
                                    name="xt", tag="xt")
                    # input prefetch on ACT HWDGE ring, separate from the
                    # output DMAs on the SP ring
                    nc.scalar.dma_start(xt[:],
                                        xt_d[g].rearrange("m p c -> p m c"))
                    if rep == 0 and g == 0:
                        # after the first input group so they don't delay it
                        nc.sync.dma_start(wp[:], wpq_d[:, 0:512])
                        nc.sync.dma_start(wq[:], wpq_d[:, 512:1024])
                    for half in range(CHUNKS_PER_GROUP):
                        r = g * CHUNKS_PER_GROUP + half
                        cs = slice(half * 128, (half + 1) * 128)
                        chunk_body(r, lambda m, cs=cs: xt[:, m, cs],
                                   r == CHUNKS - 1)
    nc.compile()
    return nc


_NC_CACHE = {}


def _get_nc(reps=1):
    if reps not in _NC_CACHE:
        _NC_CACHE[reps] = _build_nc(reps)
    return _NC_CACHE[reps]


def _prep_core_input(xb):
    """xb: [C, H, W] float32 -> dict of device input arrays (bf16).

    xt[g, m, p, c] = X[128m+p, 256g+c] where X = x.T zero-padded by 64."""
    rows = np.ascontiguousarray(xb.reshape(ROWS, W))
    X = np.zeros((MT * 128, ROWS), dtype=COMPUTE_NP)
    X[64:64 + W, :] = rows.T.astype(COMPUTE_NP)
    xt = X.reshape(MT, 128, GROUPS, GROUP_ROWS)
    return {"xt": np.ascontiguousarray(xt.transpose(2, 0, 1, 3))}


def _in_maps(x):
    WP, WQ = _weights()
    wpq = np.ascontiguousarray(np.concatenate([WP, WQ], axis=1))
    return [dict(_prep_core_input(x[b]), wpq=wpq) for b in range(N_CORES)]


def _postprocess(out_dev):
    """out_dev: [CHUNKS, 128, JT*S*128] bf16 -> [C, S, H, W] float32."""
    o = np.asarray(out_dev).astype(np.float32)
    o = o.reshape(C, 128, JT, S, 128).transpose(0, 3, 1, 2, 4)
    return o.reshape(C, S, H, W)


def kernel(x):
    x = np.asarray(x, dtype=np.float32)
    assert x.shape == (B, C, H, W)
    in_maps = _in_maps(x)
    nc = _get_nc()
    res = run_bass_kernel_spmd(nc, in_maps, core_ids=list(range(N_CORES)))
    out = np.stack([_postprocess(res.results[b]["out"]) for b in range(N_CORES)])
    return out  # [B, C, S, H, W] float32
